# revision 1
# baseline (speedup 1.0000x reference)
"""Trainium2 Bass kernel for nn_AttnGreedySearch (attn greedy top-1 search).

Math restructure (exact in exact arithmetic):
  With A_t = W_k^t and c_t = b_k @ sum_{i<t} W_k^i (row form), the iterated
  corpus is ic_t = ic0 @ A_t + c_t where ic0 = X @ W_proj + b_proj.  Define
  the per-sample query column u~_j = A_{j+1} @ S_j with S_j = user + sum v_i
  (unnormalized running sum; positive scale + constant shift never change the
  argmax; softmax is monotonic so it is argmax-irrelevant).  Per iteration j:
      score'_j[s] = <ic0[s, :], u~_j>          (argmax-equivalent scores)
      g_j = ic0[argmax]                         (one-hot select, 16-dim)
      v_j = A_{j+1}^T g_j + c_{j+1}             (output row, exact)
      u~_{j+1} = W_k u~_j + M_j g_j + d_j,      M_j = A_{j+2} A_{j+1}^T,
                                                d_j = A_{j+2} c_{j+1}
  The 262MB corpus is read once, compressed 6.25x by the projection, and the
  whole recurrence runs on 16-dim per-sample state.

Device dataflow (per core, batch B; super-tiles ST of 512 samples; P2/P3
groups of 1024 samples = 8 consecutive 128-sample tiles):
  P1a: DMA X tile [128,1000]; PE-transpose per item block -> psum [100,640];
       fat strided copy assembles s-major xt_sb [101, 10*512] whose row 100
       is all-ones (bias folded into the matmul as an extra contraction row).
  P1b: projection in B layout: matmul(lhsT=[W_proj;b_proj] [101,16],
       rhs=xt_sb s-slice [101,512]) -> psum pairs [32,512] (s even/odd rows).
       DVE 32x32 stream-transposes turn the pairs directly into sample-major
       ic0a [128, (s,h) per tile]; GPSIMD strided copies derive ic0b [(h,s)].
  P2:  per group, 5 chained iterations: score = mul+grouped-reduce (DVE),
       one-hot mask (DVE), masked select-sum (GPSIMD mul + DVE reduce),
       16x16 recurrence as 128x128 block-diagonal matmuls (PE) with
       PE transposes between sample-major and feature-major layouts; pure
       copies ride the Scalar engine.  Groups overlap P1 of later tiles.
  P3:  per tile, one [128,96] DMA stores [user | v_0..v_4] rows.
"""

import numpy as np

import concourse.bass as bass
import concourse.mybir as mybir
import concourse.tile as tile
from concourse import bacc
from concourse.bass_utils import run_bass_kernel_spmd
from concourse.masks import make_identity

F32 = mybir.dt.float32
SEARCH_NUM = 5
NCORES = 8
D = 100   # item feature dim
NSI = 10  # items per sample
H = 16    # projected dim
SH = NSI * H  # 160


def _host_constants(W_proj, b_proj, W_k, b_k):
    Wk = W_k.astype(np.float64)
    bk = b_k.astype(np.float64)
    A = [np.eye(H)]
    for _ in range(SEARCH_NUM + 1):
        A.append(A[-1] @ Wk)
    c = [np.zeros(H)]
    for _ in range(SEARCH_NUM + 1):
        c.append(c[-1] @ Wk + bk)

    def blkdiag8(m):
        out = np.zeros((128, 128))
        for t in range(8):
            out[t * H:(t + 1) * H, t * H:(t + 1) * H] = m
        return out.astype(np.float32)

    cst = {"blk_wk": blkdiag8(Wk.T)}
    for j in range(SEARCH_NUM):
        Aj1 = A[j + 1]
        cst[f"blk_a{j}"] = blkdiag8(Aj1)
        cst[f"cv{j}"] = np.tile(c[j + 1], 8).astype(np.float32)[:, None]
        if j < SEARCH_NUM - 1:
            Mj = A[j + 2] @ Aj1.T
            cst[f"blk_m{j}"] = blkdiag8(Mj.T)
            dj = A[j + 2] @ c[j + 1]
            cst[f"dv{j}"] = np.tile(dj, 8).astype(np.float32)[:, None]
    waug = np.zeros((D + 1, 32), dtype=np.float32)
    waug[:D, :H] = W_proj.astype(np.float32)
    waug[D, :H] = b_proj.astype(np.float32)
    cst["waug"] = waug
    return cst


def _v(t, off, dims, nparts=None):
    """View on tile/AP t: free dims `dims`, element offset `off` added.
    `nparts` overrides the partition count (step preserved)."""
    p = list(t.ap[0])
    if nparts is not None:
        p = [p[0], nparts]
    return bass.AP(tensor=t.tensor, offset=t.offset + off,
                   ap=[p] + [list(d) for d in dims])


def build_program(nc, B):
    assert B % 1024 == 0
    NT = B // 128
    NST = B // 512
    NG = B // 1024
    dt = F32

    x_d = nc.dram_tensor("x", [B, NSI, D], dt, kind="ExternalInput").ap()
    user_d = nc.dram_tensor("user", [B, H], dt, kind="ExternalInput").ap()
    waug_d = nc.dram_tensor("waug", [D + 1, 32], dt, kind="ExternalInput").ap()
    ones_d = nc.dram_tensor("ones_row", [1, NSI * 512], dt,
                            kind="ExternalInput").ap()
    blk_wk_d = nc.dram_tensor("blk_wk", [128, 128], dt, kind="ExternalInput").ap()
    blk_a_d = [nc.dram_tensor(f"blk_a{j}", [128, 128], dt, kind="ExternalInput").ap()
               for j in range(SEARCH_NUM)]
    cv_d = [nc.dram_tensor(f"cv{j}", [128, 1], dt, kind="ExternalInput").ap()
            for j in range(SEARCH_NUM)]
    blk_m_d = [nc.dram_tensor(f"blk_m{j}", [128, 128], dt, kind="ExternalInput").ap()
               for j in range(SEARCH_NUM - 1)]
    dv_d = [nc.dram_tensor(f"dv{j}", [128, 1], dt, kind="ExternalInput").ap()
            for j in range(SEARCH_NUM - 1)]
    out_d = nc.dram_tensor("out", [B, SEARCH_NUM + 1, H], dt,
                           kind="ExternalOutput").ap()

    with tile.TileContext(nc) as tc:
        with tc.tile_pool(name="singles", bufs=1) as singles, \
             tc.tile_pool(name="xst", bufs=3) as xst, \
             tc.tile_pool(name="xtp", bufs=2) as xtp, \
             tc.tile_pool(name="scr", bufs=3) as scr, \
             tc.tile_pool(name="vop", bufs=3) as vop, \
             tc.tile_pool(name="ptx", bufs=3, space="PSUM") as ptx, \
             tc.tile_pool(name="ppr", bufs=2, space="PSUM") as ppr, \
             tc.tile_pool(name="pp2", bufs=3, space="PSUM") as pp2:

            # ---- persistent SBUF ----
            ident = singles.tile([128, 128], dt)
            make_identity(nc, ident)
            waug_sb = singles.tile([D + 1, 32], dt)
            nc.sync.dma_start(out=waug_sb, in_=waug_d)
            blk_wk_sb = singles.tile([128, 128], dt)
            nc.sync.dma_start(out=blk_wk_sb, in_=blk_wk_d)
            blk_a_sb, cv_sb, blk_m_sb, dv_sb = [], [], [], []
            for j in range(SEARCH_NUM):
                t_ = singles.tile([128, 128], dt, name=f"blk_a{j}_sb")
                nc.sync.dma_start(out=t_, in_=blk_a_d[j])
                blk_a_sb.append(t_)
                t_ = singles.tile([128, 1], dt, name=f"cv{j}_sb")
                nc.sync.dma_start(out=t_, in_=cv_d[j])
                cv_sb.append(t_)
            for j in range(SEARCH_NUM - 1):
                t_ = singles.tile([128, 128], dt, name=f"blk_m{j}_sb")
                nc.sync.dma_start(out=t_, in_=blk_m_d[j])
                blk_m_sb.append(t_)
                t_ = singles.tile([128, 1], dt, name=f"dv{j}_sb")
                nc.sync.dma_start(out=t_, in_=dv_d[j])
                dv_sb.append(t_)

            ic0a = singles.tile([128, NT * SH], dt)   # (s,h) per tile
            ic0b = singles.tile([128, NT * SH], dt)   # (h,s) per tile
            usera = singles.tile([128, NG * 128], dt)  # group-major (t,h)
            ua = singles.tile([128, NG * 128], dt)     # u~ sample-major
            ud = singles.tile([128, NG * 128], dt)     # u~ feature-major

            # ---- P0: user load + u~_0 = W_k @ user, per group ----
            for g in range(NG):
                src_ap = bass.AP(
                    tensor=user_d.tensor,
                    offset=user_d.offset + g * 1024 * H,
                    ap=[[H, 128], [128 * H, 8], [1, H]],
                )
                nc.sync.dma_start(out=usera[:, g * 128:(g + 1) * 128],
                                  in_=src_ap)
                tp = pp2.tile([128, 128], dt, name="tp0", tag="p2")
                nc.tensor.transpose(tp, usera[:, g * 128:(g + 1) * 128], ident)
                userd_g = scr.tile([128, 128], dt, name="userd_g", tag="gd")
                nc.scalar.copy(userd_g, tp)
                up = pp2.tile([128, 128], dt, name="up0", tag="p2")
                nc.tensor.matmul(up, blk_wk_sb, userd_g, start=True, stop=True)
                nc.scalar.copy(ud[:, g * 128:(g + 1) * 128], up)
                tp2 = pp2.tile([128, 128], dt, name="tp0b", tag="p2")
                nc.tensor.transpose(tp2, ud[:, g * 128:(g + 1) * 128], ident)
                nc.scalar.copy(ua[:, g * 128:(g + 1) * 128], tp2)

            # ---- main loop over super-tiles, P2/P3 interleaved ----
            for st in range(NST):
                # --- P1a ---
                xt_sb = xtp.tile([D + 1, NSI * 512], dt, name="xt_sb")
                nc.sync.dma_start(out=xt_sb[D:D + 1, :], in_=ones_d)
                for a in range(4):
                    c = st * 4 + a
                    xstage = xst.tile([128, NSI * D], dt, name="xstage")
                    nc.sync.dma_start(out=xstage,
                                      in_=x_d[c * 128:(c + 1) * 128, :, :])
                    for gi, (s0, ns_) in enumerate(((0, 4), (4, 4), (8, 2))):
                        tp = ptx.tile([D, 512], dt, name="tpx", tag="tpx")
                        for k in range(ns_):
                            s = s0 + k
                            nc.tensor.transpose(
                                tp[:, k * 128:(k + 1) * 128],
                                xstage[:, s * D:(s + 1) * D],
                                ident)
                        dst = _v(xt_sb, s0 * 512 + a * 128,
                                 [[512, ns_], [1, 128]], nparts=D)
                        srcv = _v(tp, 0, [[128, ns_], [1, 128]])
                        if (a + gi) % 2 == 0:
                            nc.vector.tensor_copy(dst, srcv)
                        else:
                            nc.scalar.copy(dst, srcv)
                # --- P1b: projection (3 items per psum tile, bases
                #     0/32/64) + PE-based B->A assembly ---
                for ss in ((0, 1, 2), (3, 4, 5), (6, 7, 8), (9,)):
                    ns = len(ss)
                    pp = ppr.tile([128, 512], dt, name="pp", tag="pp")
                    for a_mm in range(4):
                        for k, s in enumerate(ss):
                            out_mm = _v(pp,
                                        32 * k * pp.ap[0][0] + a_mm * 128,
                                        [[1, 128]], nparts=32)
                            nc.tensor.matmul(
                                out_mm, waug_sb,
                                xt_sb[:, s * 512 + a_mm * 128:
                                      s * 512 + (a_mm + 1) * 128],
                                start=True, stop=True)
                    pps4 = scr.tile([32 * ns, 512], dt, name="pps4",
                                    tag="pps")
                    nc.scalar.copy(pps4, _v(pp, 0, [[1, 512]],
                                            nparts=32 * ns))
                    for a in range(4):
                        c = st * 4 + a
                        tpb = ppr.tile([128, 32 * ns], dt, name="tpb",
                                       tag="pp")
                        nc.tensor.transpose(
                            tpb, pps4[:, a * 128:(a + 1) * 128],
                            ident[:32 * ns, :32 * ns])
                        nc.scalar.copy(
                            _v(ic0a, c * SH + ss[0] * H, [[H, ns], [1, H]]),
                            _v(tpb, 0, [[32, ns], [1, H]]))
                # --- ic0b (h,s) via GPSIMD strided copies ---
                for a in range(4):
                    c = st * 4 + a
                    nc.gpsimd.tensor_copy(
                        ic0b[:, c * SH:(c + 1) * SH],
                        _v(ic0a, c * SH, [[1, H], [H, NSI]]))

                # --- P2 + P3 for completed group ---
                if st % 2 == 1:
                    g = st // 2
                    _emit_group(nc, g, ident, ic0a, ic0b, usera, ua, ud,
                                blk_wk_sb, blk_a_sb, cv_sb, blk_m_sb, dv_sb,
                                scr, vop, pp2, out_d, dt)


def _emit_group(nc, g, ident, ic0a, ic0b, usera, ua, ud,
                blk_wk_sb, blk_a_sb, cv_sb, blk_m_sb, dv_sb,
                scr, vop, pp2, out_d, dt):
    base = g * 8 * SH
    ua_sl = _v(ua, g * 128, [[H, 8], [0, NSI], [1, H]])
    vout = vop.tile([128, 8 * 96], dt, name="vout")
    for j in range(SEARCH_NUM):
        prod = scr.tile([128, 8, NSI, H], dt, name="prod", tag="prod")
        nc.vector.tensor_tensor(
            out=prod,
            in0=_v(ic0a, base, [[SH, 8], [H, NSI], [1, H]]),
            in1=ua_sl, op=mybir.AluOpType.mult)
        scores = scr.tile([128, 8, NSI], dt, name="scores", tag="scores")
        nc.vector.reduce_sum(out=scores, in_=prod, axis=mybir.AxisListType.X)
        mx = scr.tile([128, 8], dt, name="mx", tag="mx")
        nc.vector.reduce_max(out=mx, in_=scores, axis=mybir.AxisListType.X)
        mask = scr.tile([128, 8, NSI], dt, name="mask", tag="mask")
        nc.vector.tensor_tensor(
            out=mask, in0=scores, in1=_v(mx, 0, [[1, 8], [0, NSI]]),
            op=mybir.AluOpType.is_equal)
        sel = scr.tile([128, 8, H, NSI], dt, name="sel", tag="sel")
        nc.vector.tensor_tensor(
            out=sel,
            in0=_v(ic0b, base, [[SH, 8], [NSI, H], [1, NSI]]),
            in1=_v(mask, 0, [[NSI, 8], [0, H], [1, NSI]]),
            op=mybir.AluOpType.mult)
        ga = scr.tile([128, 8, H], dt, name="ga", tag="ga")
        nc.vector.reduce_sum(out=ga, in_=sel, axis=mybir.AxisListType.X)
        tpg = pp2.tile([128, 128], dt, name="tpg", tag="p2")
        nc.tensor.transpose(tpg, ga, ident)
        gd_g = scr.tile([128, 128], dt, name="gd_g", tag="gd")
        nc.scalar.copy(gd_g, tpg)
        # u~ recurrence first: it is the cross-iteration critical path
        if j < SEARCH_NUM - 1:
            up = pp2.tile([128, 128], dt, name="upj", tag="p2")
            ud_sl = ud[:, g * 128:(g + 1) * 128]
            nc.tensor.matmul(up, blk_wk_sb, ud_sl, start=True, stop=False)
            nc.tensor.matmul(up, blk_m_sb[j], gd_g, start=False, stop=True)
            nc.vector.tensor_scalar_add(ud_sl, up, dv_sb[j])
            tpu = pp2.tile([128, 128], dt, name="tpu", tag="p2")
            nc.tensor.transpose(tpu, ud_sl, ident)
            nc.scalar.copy(ua[:, g * 128:(g + 1) * 128], tpu)
        vp = pp2.tile([128, 128], dt, name="vp", tag="p2")
        nc.tensor.matmul(vp, blk_a_sb[j], gd_g, start=True, stop=True)
        vtmp = scr.tile([128, 128], dt, name="vtmp", tag="vtmp")
        nc.vector.tensor_scalar_add(vtmp, vp, cv_sb[j])
        tpv = pp2.tile([128, 128], dt, name="tpv", tag="p2")
        nc.tensor.transpose(tpv, vtmp, ident)
        nc.scalar.copy(_v(vout, (1 + j) * H, [[96, 8], [1, H]]), tpv)
    nc.scalar.copy(_v(vout, 0, [[96, 8], [1, H]]),
                   _v(usera, g * 128, [[H, 8], [1, H]]))
    for t in range(8):
        c = g * 8 + t
        nc.sync.dma_start(
            out=out_d[c * 128:(c + 1) * 128, :, :],
            in_=vout[:, t * 96:(t + 1) * 96].rearrange("p (j h) -> p j h", j=6))


def _in_maps(inputs, B_core):
    cst = _host_constants(inputs["W_proj"], inputs["b_proj"],
                          inputs["W_k"], inputs["b_k"])
    x = np.ascontiguousarray(inputs["item_corpus"], dtype=np.float32)
    u = np.ascontiguousarray(inputs["user_intent"], dtype=np.float32)
    ones = np.ones((1, NSI * 512), dtype=np.float32)
    maps = []
    for core in range(NCORES):
        lo, hi = core * B_core, (core + 1) * B_core
        m = {"x": x[lo:hi], "user": u[lo:hi], "waug": cst["waug"],
             "ones_row": ones, "blk_wk": cst["blk_wk"]}
        for j in range(SEARCH_NUM):
            m[f"blk_a{j}"] = cst[f"blk_a{j}"]
            m[f"cv{j}"] = cst[f"cv{j}"]
            if j < SEARCH_NUM - 1:
                m[f"blk_m{j}"] = cst[f"blk_m{j}"]
                m[f"dv{j}"] = cst[f"dv{j}"]
        maps.append(m)
    return maps


_COMPILED = {}


def _get_nc(B_core):
    if B_core not in _COMPILED:
        nc = bacc.Bacc("TRN2", target_bir_lowering=False, debug=False,
                       num_devices=NCORES)
        build_program(nc, B_core)
        nc.compile()
        _COMPILED[B_core] = nc
    return _COMPILED[B_core]


def kernel(**inputs) -> np.ndarray:
    bs = inputs["user_intent"].shape[0]
    assert bs % NCORES == 0
    B_core = bs // NCORES
    nc = _get_nc(B_core)
    res = run_bass_kernel_spmd(nc, _in_maps(inputs, B_core),
                               core_ids=list(range(NCORES)))
    out = np.concatenate([r["out"] for r in res.results], axis=0)
    return out.astype(np.float32)

